# revision 6
# baseline (speedup 1.0000x reference)
"""Trainium2 Bass kernel for nn_MultiHeadAttention_72189810312078.

Computation (per token): qkv = x @ w_qkv.T + b_qkv; per-token attention over
the 16 heads with 16x16 score matrices; out = attn_out @ w_out.T + b_out.

Strategy: data-parallel over 8 NeuronCores (8192 tokens each). Host
pre-transposes x to xT [1024, N] so the channel (contraction) dim lands on
SBUF partitions. Per core, per 256-token superblock:
  1) qkvT projection: 24 feature-chunk matmuls (N=256), K=1024 accumulated in
     PSUM; per-partition bias added in PSUM with one tensor_scalar_add.
  2) PSUM chunks scatter-evicted into attention staging: Q at partitions
     64:128 of T1, K at 64:128 / V at 0:64 of T2 (matmul operands need equal
     base partitions), laid out [d, (group, head, t)].
  3) Attention in groups of 8 tokens ((g,t) packs 16x8=128 partitions):
     scoresT = K.T @ Q per group (K=64 matmul at tile_position row 64);
     exp on ScalarE; multiplicative block-diagonal mask; V8 =
     PE-transpose(V); attnV matmul with a ones column appended to V8 so the
     softmax denominator falls out of the same matmul; normalize with a
     per-partition reciprocal scale on eviction.
  4) attn output PE-transposed back to feature-major, packed into S2
     [128 = (dlt,d), chunk x token]; out-projection against host-permuted
     w_out.T rows (feature 64*(8*dlt+c)+d at S2 row 128c+64*dlt+d); bias
     added from a replicated tile during eviction; result DMA'd row-major.

Dtype modes (KMODE): "bf16" (default) runs every matmul operand in bfloat16
(1 cycle/row on the PE at any moving size, FWL weight loads); "f32r" keeps
fp32-rounded (11-bit mantissa) operands for the projections (1 cycle/row only
at moving>=256) with fp32 attention staging. PSUM accumulation is fp32 always.
"""

import os
import sys
from contextlib import ExitStack, nullcontext

sys.path.insert(0, "/opt/trn_rl_repo")

import numpy as np
import ml_dtypes

import concourse.bass as bass  # noqa: E402
import concourse.bacc as bacc  # noqa: E402
import concourse.tile as tile  # noqa: E402
from concourse import mybir  # noqa: E402
from concourse.bass_utils import run_bass_kernel_spmd  # noqa: E402
from concourse.masks import make_identity  # noqa: E402

F32 = mybir.dt.float32
F32R = mybir.dt.float32r
BF16 = mybir.dt.bfloat16

N_CORES = 8
H, D, C = 16, 64, 1024
SB = 256   # tokens per superblock (projection moving dim)
SS = 128   # tokens per attention sub-stage / out-projection block
NG = SB // 8   # token groups per superblock (32)

KMODE = os.environ.get("KMODE", "bf16")
GPS = int(os.environ.get("GPS", "3"))  # bitmask: 1=memset, 2=mask-mul, 4=bias
Exp = mybir.ActivationFunctionType.Exp
Copy = mybir.ActivationFunctionType.Copy


def _dtypes(mode):
    """-> (WD projection-weight/x dtype, SD attention staging, AD S2)."""
    if mode == "bf16":
        return BF16, BF16, BF16
    if mode == "f32r":
        return F32R, F32, F32R
    return F32, F32, F32


def build(tok, mode=KMODE, static_loop=False):
    WD, SD, AD = _dtypes(mode)

    nc = bacc.Bacc("TRN2", target_bir_lowering=False, debug=False,
                   enable_asserts=True, num_devices=N_CORES)
    xT_d = nc.dram_tensor("xT", [C, tok], WD, kind="ExternalInput").ap()
    wqkvT_d = nc.dram_tensor("wqkvT", [C, 3 * C], WD, kind="ExternalInput").ap()
    woutT_d = nc.dram_tensor("woutT", [C, C], WD, kind="ExternalInput").ap()
    bcols_d = nc.dram_tensor("bcols", [128, 24], F32, kind="ExternalInput").ap()
    borep_d = nc.dram_tensor("borep", [128, C], F32, kind="ExternalInput").ap()
    maskB_d = nc.dram_tensor("maskB", [128, 512], SD, kind="ExternalInput").ap()
    out_d = nc.dram_tensor("out", [tok, C], F32, kind="ExternalOutput").ap()

    with tile.TileContext(nc) as tc, ExitStack() as ctx:
        consts = ctx.enter_context(tc.tile_pool(name="consts", bufs=1))
        xin = ctx.enter_context(tc.tile_pool(name="xin", bufs=2))
        stag = ctx.enter_context(tc.tile_pool(name="stag", bufs=1))
        smx = ctx.enter_context(tc.tile_pool(name="smx", bufs=2))
        s2p = ctx.enter_context(tc.tile_pool(name="s2p", bufs=2))
        outp = ctx.enter_context(tc.tile_pool(name="outp", bufs=2))
        psA = ctx.enter_context(tc.tile_pool(name="psA", bufs=2, space="PSUM"))
        psSp = ctx.enter_context(tc.tile_pool(name="psSp", bufs=2, space="PSUM"))
        psVp = ctx.enter_context(tc.tile_pool(name="psVp", bufs=1, space="PSUM"))
        psC2p = ctx.enter_context(tc.tile_pool(name="psC2p", bufs=1, space="PSUM"))
        psTp = ctx.enter_context(tc.tile_pool(name="psTp", bufs=1, space="PSUM"))
        psOp = ctx.enter_context(tc.tile_pool(name="psOp", bufs=1, space="PSUM"))

        # ---- constants ----
        wq_sb = consts.tile([128, 8, 3 * C], WD)
        nc.sync.dma_start(out=wq_sb, in_=wqkvT_d.rearrange("(ci p) f -> p ci f", p=128))
        wo_sb = consts.tile([128, 8, C], WD)
        nc.sync.dma_start(out=wo_sb, in_=woutT_d.rearrange("(ci p) f -> p ci f", p=128))
        bcols_sb = consts.tile([128, 24], F32)
        nc.sync.dma_start(out=bcols_sb, in_=bcols_d)
        borep_sb = consts.tile([128, C], F32)
        nc.sync.dma_start(out=borep_sb, in_=borep_d)
        maskB_sb = consts.tile([128, 512], SD)
        nc.sync.dma_start(out=maskB_sb, in_=maskB_d)
        idq = consts.tile([128, 128], F32)
        make_identity(nc, idq)
        if SD is F32:
            idS = idq
        else:
            idS = consts.tile([128, 128], SD)
            nc.vector.tensor_copy(idS, idq)
        if AD is F32:
            idr = idq
        elif AD is SD:
            idr = idS
        else:
            idr = consts.tile([128, 128], AD)
            nc.vector.tensor_copy(idr, idq)

        ecnt = 0  # evict-engine round robin

        def evict_copy(dst, src):
            nonlocal ecnt
            if ecnt % 2 == 0:
                nc.vector.tensor_copy(dst, src)
            else:
                nc.scalar.copy(dst, src)
            ecnt += 1

        xT_r = xT_d.rearrange("(ci p) t -> p ci t", p=128)
        if static_loop:
            loop_iter = [(nullcontext(iv), iv) for iv in range(0, tok, SB)]
        else:
            fc = tc.For_i(0, tok, SB,
                          hint_engines=(mybir.EngineType.PE,
                                        mybir.EngineType.DVE))
            loop_iter = [(fc, None)]
        for _ctx, _iv in loop_iter:
          with _ctx as _cv:
            iv = _iv if _iv is not None else _cv
            x_sb = xin.tile([128, 8, SB], WD)
            nc.sync.dma_start(out=x_sb, in_=xT_r[:, :, bass.ds(iv, SB)])

            # staging: T1 rows 64:128 = Q; T2 rows 64:128 = K, rows 0:64 = V
            T1 = stag.tile([128, NG, 16, 8], SD, name="T1")
            T2 = stag.tile([128, NG, 16, 8], SD, name="T2")

            # ---- qkv projection + scatter-evict (bias fused / on gpsimd) ----
            for co in range(24):
                psC1 = psA.tile([128, SB], F32)
                for ci in range(8):
                    nc.tensor.matmul(psC1, wq_sb[:, ci, co * 128:(co + 1) * 128],
                                     x_sb[:, ci, :], start=(ci == 0),
                                     stop=(ci == 7))
                kind, c = co // 8, co % 8
                for dlt in range(2):
                    src = psC1[64 * dlt:64 * dlt + 64, :].rearrange(
                        "p (g t) -> p g t", g=NG)
                    hslot = 2 * c + dlt
                    if kind == 0:
                        dst = T1[64:128, :, hslot, :]
                    elif kind == 1:
                        dst = T2[64:128, :, hslot, :]
                    else:
                        dst = T2[0:64, :, hslot, :]
                    bias = bcols_sb[64 * dlt:64 * dlt + 64, co:co + 1]
                    if dlt == 0:
                        # DVE evict with fused bias add
                        nc.vector.tensor_scalar_add(dst, src, bias)
                    elif GPS & 4:
                        # ACT plain evict, bias added SBUF-side on idle gpsimd
                        nc.scalar.copy(dst, src)
                        nc.gpsimd.tensor_scalar_add(dst, dst, bias)
                    else:
                        nc.vector.tensor_scalar_add(dst, src, bias)

            # ---- attention (8 batches of 4 groups) + out-proj per 128 tok ----
            for iss in range(2):
                S2 = s2p.tile([128, 8, SS], AD)
                for b4 in range(4 * iss, 4 * iss + 4):
                    psS = psSp.tile([128, 512], F32)
                    psV = psVp.tile([128, 4, 64], SD)
                    for j in range(4):
                        g = 4 * b4 + j
                        nc.tensor.matmul(psS[:, 128 * j:128 * j + 128],
                                         T2[64:128, g, :, :], T1[64:128, g, :, :],
                                         start=True, stop=True)
                        nc.tensor.transpose(psV[:, j, :], T2[0:64, g, :, :],
                                            idS[0:64, 0:64])
                    es4 = smx.tile([128, 512], SD)
                    nc.scalar.activation(es4, psS, Exp, scale=0.125)
                    if GPS & 2:
                        nc.gpsimd.tensor_mul(es4, es4, maskB_sb)
                    else:
                        nc.vector.tensor_mul(es4, es4, maskB_sb)
                    V8sb = smx.tile([128, 4, 66], SD)
                    nc.scalar.copy(V8sb[:, :, 0:64], psV)
                    if GPS & 1:
                        nc.gpsimd.memset(V8sb[:, :, 64:65], 1.0)
                    else:
                        nc.vector.memset(V8sb[:, :, 64:65], 1.0)
                    psC2 = psC2p.tile([128, 4, 66], F32)
                    for j in range(4):
                        nc.tensor.matmul(psC2[:, j, 0:65],
                                         es4[:, 128 * j:128 * j + 128],
                                         V8sb[:, j, 0:65], start=True, stop=True)
                    rec4 = smx.tile([128, 4], F32)
                    nc.vector.reciprocal(rec4, psC2[:, :, 64:65])
                    attno = smx.tile([128, 4, 64], AD)
                    for j in range(4):
                        if j % 2 == 0:
                            nc.scalar.activation(attno[:, j, :], psC2[:, j, 0:64],
                                                 Copy, scale=rec4[:, j:j + 1])
                        else:
                            nc.vector.tensor_scalar_mul(attno[:, j, :],
                                                        psC2[:, j, 0:64],
                                                        rec4[:, j:j + 1])
                    psT = psTp.tile([64, 4, 128], AD)
                    for j in range(4):
                        nc.tensor.transpose(psT[:, j, :], attno[:, j, :], idr)
                    # S2 pack: head slots 8*dlt..8*dlt+7 -> S2 rows 64*dlt+d
                    for dlt in range(2):
                        src = psT[:, :, 64 * dlt:64 * dlt + 64].rearrange(
                            "p j (h t) -> p j h t", h=8)
                        dst = S2[64 * dlt:64 * dlt + 64].rearrange(
                            "p c (gb gj t) -> p gj c gb t", gb=4, gj=4)[
                                :, :, :, b4 % 4, :]
                        evict_copy(dst, src)

                # ---- out-projection for this 128-token block ----
                outsb = outp.tile([128, C], F32)
                for nh in range(2):
                    psO = psOp.tile([128, 512], F32)
                    for c in range(8):
                        nc.tensor.matmul(psO, S2[:, c, :],
                                         wo_sb[:, c, 512 * nh:512 * nh + 512],
                                         start=(c == 0), stop=(c == 7))
                    nc.vector.tensor_add(outsb[:, 512 * nh:512 * nh + 512], psO,
                                         borep_sb[:, 512 * nh:512 * nh + 512])
                nc.sync.dma_start(out=out_d[bass.ds(iv + SS * iss, SS), :],
                                  in_=outsb)

    nc.compile()
    return nc


def build_pipe(tok, mode=KMODE, static_loop=False):
    """Software-pipelined build: attention of superblock k overlaps the
    projection of superblock k+1 inside one For_i body (2 superblocks per
    iteration, ping-pong staging halves packed into shared tiles)."""
    WD, SD, AD = _dtypes(mode)

    nc = bacc.Bacc("TRN2", target_bir_lowering=False, debug=False,
                   enable_asserts=True, num_devices=N_CORES)
    xT_d = nc.dram_tensor("xT", [C, tok], WD, kind="ExternalInput").ap()
    wqkvT_d = nc.dram_tensor("wqkvT", [C, 3 * C], WD, kind="ExternalInput").ap()
    woutT_d = nc.dram_tensor("woutT", [C, C], WD, kind="ExternalInput").ap()
    bcols_d = nc.dram_tensor("bcols", [128, 24], F32, kind="ExternalInput").ap()
    borep_d = nc.dram_tensor("borep", [128, C], F32, kind="ExternalInput").ap()
    maskB_d = nc.dram_tensor("maskB", [128, 512], SD, kind="ExternalInput").ap()
    out_d = nc.dram_tensor("out", [tok, C], F32, kind="ExternalOutput").ap()

    PB = [int(v) for v in os.environ.get("PB", "2,2,2,2,2,2").split(",")]
    with tile.TileContext(nc) as tc, ExitStack() as ctx:
        consts = ctx.enter_context(tc.tile_pool(name="consts", bufs=1))
        xin = ctx.enter_context(tc.tile_pool(name="xin", bufs=PB[0]))
        stag = ctx.enter_context(tc.tile_pool(name="stag", bufs=1))
        smx = ctx.enter_context(tc.tile_pool(name="smx", bufs=PB[1]))
        s2p = ctx.enter_context(tc.tile_pool(name="s2p", bufs=PB[2]))
        outp = ctx.enter_context(tc.tile_pool(name="outp", bufs=PB[3]))
        psA = ctx.enter_context(tc.tile_pool(name="psA", bufs=PB[4], space="PSUM"))
        psSp = ctx.enter_context(tc.tile_pool(name="psSp", bufs=PB[5], space="PSUM"))
        psVp = ctx.enter_context(tc.tile_pool(name="psVp", bufs=1, space="PSUM"))
        psC2p = ctx.enter_context(tc.tile_pool(name="psC2p", bufs=1, space="PSUM"))
        psTp = ctx.enter_context(tc.tile_pool(name="psTp", bufs=1, space="PSUM"))
        psOp = ctx.enter_context(tc.tile_pool(name="psOp", bufs=1, space="PSUM"))

        wq_sb = consts.tile([128, 8, 3 * C], WD)
        nc.sync.dma_start(out=wq_sb, in_=wqkvT_d.rearrange("(ci p) f -> p ci f", p=128))
        wo_sb = consts.tile([128, 8, C], WD)
        nc.sync.dma_start(out=wo_sb, in_=woutT_d.rearrange("(ci p) f -> p ci f", p=128))
        bcols_sb = consts.tile([128, 24], F32)
        nc.sync.dma_start(out=bcols_sb, in_=bcols_d)
        borep_sb = consts.tile([128, C], F32)
        nc.sync.dma_start(out=borep_sb, in_=borep_d)
        maskB_sb = consts.tile([128, 512], SD)
        nc.sync.dma_start(out=maskB_sb, in_=maskB_d)
        idq = consts.tile([128, 128], F32)
        make_identity(nc, idq)
        if SD is F32:
            idS = idq
        else:
            idS = consts.tile([128, 128], SD)
            nc.vector.tensor_copy(idS, idq)
        if AD is F32:
            idr = idq
        elif AD is SD:
            idr = idS
        else:
            idr = consts.tile([128, 128], AD)
            nc.vector.tensor_copy(idr, idq)

        # persistent ping-pong staging (half-set hb=0: Q/K upper, V lower)
        Q_AB = stag.tile([128, NG, 16, 8], SD, name="Q_AB")
        K_AB = stag.tile([128, NG, 16, 8], SD, name="K_AB")
        V_AB = stag.tile([128, NG, 16, 8], SD, name="V_AB")

        xT_r = xT_d.rearrange("(ci p) t -> p ci t", p=128)

        def emit_xload(piv):
            x_sb = xin.tile([128, 8, SB], WD)
            nc.sync.dma_start(out=x_sb, in_=xT_r[:, :, bass.ds(piv, SB)])
            return x_sb

        def qk_half(T, hb):
            return T[64 * (1 - hb):64 * (1 - hb) + 64]

        def v_half(hb):
            return V_AB[64 * hb:64 * hb + 64]

        def emit_proj_chunk(x_sb, co, hb):
            psC1 = psA.tile([128, SB], F32)
            for ci in range(8):
                nc.tensor.matmul(psC1, wq_sb[:, ci, co * 128:(co + 1) * 128],
                                 x_sb[:, ci, :], start=(ci == 0), stop=(ci == 7))
            kind, c = co // 8, co % 8
            for dlt in range(2):
                src = psC1[64 * dlt:64 * dlt + 64, :].rearrange(
                    "p (g t) -> p g t", g=NG)
                hslot = 2 * c + dlt
                if kind == 0:
                    dst = qk_half(Q_AB, hb)[:, :, hslot, :]
                elif kind == 1:
                    dst = qk_half(K_AB, hb)[:, :, hslot, :]
                else:
                    dst = v_half(hb)[:, :, hslot, :]
                bias = bcols_sb[64 * dlt:64 * dlt + 64, co:co + 1]
                if dlt == 0:
                    nc.vector.tensor_scalar_add(dst, src, bias)
                else:
                    nc.scalar.copy(dst, src)
                    nc.gpsimd.tensor_scalar_add(dst, dst, bias)

        def emit_attn_batch1(b4, hb):
            """scores + V transposes + exp + mask for groups 4*b4..4*b4+3."""
            psS = psSp.tile([128, 512], F32)
            psV = psVp.tile([128, 4, 64], SD)
            vb = 64 * hb
            for j in range(4):
                g = 4 * b4 + j
                nc.tensor.matmul(psS[:, 128 * j:128 * j + 128],
                                 qk_half(K_AB, hb)[:, g, :, :],
                                 qk_half(Q_AB, hb)[:, g, :, :],
                                 start=True, stop=True)
                nc.tensor.transpose(psV[:, j, :], v_half(hb)[:, g, :, :],
                                    idS[vb:vb + 64, vb:vb + 64])
            es4 = smx.tile([128, 512], SD)
            nc.scalar.activation(es4, psS, Exp, scale=0.125)
            nc.gpsimd.tensor_mul(es4, es4, maskB_sb)
            V8sb = smx.tile([128, 4, 66], SD)
            nc.scalar.copy(V8sb[:, :, 0:64], psV)
            nc.gpsimd.memset(V8sb[:, :, 64:65], 1.0)
            return es4, V8sb

        def emit_attn_batch2(b4, es4, V8sb, S2):
            psC2 = psC2p.tile([128, 4, 66], F32)
            for j in range(4):
                nc.tensor.matmul(psC2[:, j, 0:65], es4[:, 128 * j:128 * j + 128],
                                 V8sb[:, j, 0:65], start=True, stop=True)
            rec4 = smx.tile([128, 4], F32)
            nc.vector.reciprocal(rec4, psC2[:, :, 64:65])
            attno = smx.tile([128, 4, 64], AD)
            for j in range(4):
                if j % 2 == 0:
                    nc.scalar.activation(attno[:, j, :], psC2[:, j, 0:64],
                                         Copy, scale=rec4[:, j:j + 1])
                else:
                    nc.vector.tensor_scalar_mul(attno[:, j, :],
                                                psC2[:, j, 0:64],
                                                rec4[:, j:j + 1])
            psT = psTp.tile([64, 4, 128], AD)
            for j in range(4):
                nc.tensor.transpose(psT[:, j, :], attno[:, j, :], idr)
            for dlt in range(2):
                src = psT[:, :, 64 * dlt:64 * dlt + 64].rearrange(
                    "p j (h t) -> p j h t", h=8)
                dst = S2[64 * dlt:64 * dlt + 64].rearrange(
                    "p c (gb gj t) -> p gj c gb t", gb=4, gj=4)[:, :, :, b4 % 4, :]
                if dlt == 0:
                    nc.vector.tensor_copy(dst, src)
                else:
                    nc.scalar.copy(dst, src)

        def emit_outproj(S2, oiv, iss):
            outsb = outp.tile([128, C], F32)
            for nh in range(2):
                psO = psOp.tile([128, 512], F32)
                for c in range(8):
                    nc.tensor.matmul(psO, S2[:, c, :],
                                     wo_sb[:, c, 512 * nh:512 * nh + 512],
                                     start=(c == 0), stop=(c == 7))
                nc.vector.tensor_add(outsb[:, 512 * nh:512 * nh + 512], psO,
                                     borep_sb[:, 512 * nh:512 * nh + 512])
            nc.sync.dma_start(out=out_d[bass.ds(oiv + SS * iss, SS), :], in_=outsb)

        def emit_part(attn_oiv, attn_hb, proj_piv, proj_hb):
            """Weave attention of one superblock with projection of another.
            Either may be None (prologue/epilogue)."""
            x_sb = emit_xload(proj_piv) if proj_piv is not None else None
            S2 = None
            for b4 in range(8):
                if attn_oiv is not None:
                    if b4 % 4 == 0:
                        S2 = s2p.tile([128, 8, SS], AD, name="S2")
                    pend = emit_attn_batch1(b4, attn_hb)
                if x_sb is not None:
                    for co in range(3 * b4, 3 * b4 + 3):
                        emit_proj_chunk(x_sb, co, proj_hb)
                if attn_oiv is not None:
                    emit_attn_batch2(b4, *pend, S2)
                    if b4 % 4 == 3:
                        emit_outproj(S2, attn_oiv, b4 // 4)

        assert tok % (2 * SB) == 0 and tok >= 2 * SB
        emit_part(None, None, 0, 0)                      # prologue: proj sb0 -> A
        if tok > 2 * SB and static_loop:
            for iv in range(0, tok - 2 * SB, 2 * SB):
                emit_part(iv, 0, iv + SB, 1)             # attn A, proj -> B
                emit_part(iv + SB, 1, iv + 2 * SB, 0)    # attn B, proj -> A
        elif tok > 2 * SB:
            with tc.For_i(0, tok - 2 * SB, 2 * SB,
                          hint_engines=(mybir.EngineType.PE, mybir.EngineType.DVE,
                                        mybir.EngineType.Activation)) as iv:
                emit_part(iv, 0, iv + SB, 1)             # attn A, proj -> B
                emit_part(iv + SB, 1, iv + 2 * SB, 0)    # attn B, proj -> A
        last = tok - 2 * SB
        emit_part(last, 0, tok - SB, 1)                  # attn A, proj last -> B
        emit_part(tok - SB, 1, None, None)               # attn B
    nc.compile()
    return nc


def _round_f32r(a):
    """Round fp32 to the f32r grid (drop 12 mantissa bits, round-to-nearest)."""
    b = np.ascontiguousarray(a, dtype=np.float32).view(np.uint32)
    b = ((b + (1 << 11)) >> 12) << 12
    return b.view(np.float32)


def _wcast(a, mode):
    if mode == "bf16":
        return np.ascontiguousarray(a.astype(ml_dtypes.bfloat16))
    if mode == "f32r":
        return _round_f32r(np.ascontiguousarray(a, dtype=np.float32))
    return np.ascontiguousarray(a, dtype=np.float32)


def _host_prep(x, w_qkv, b_qkv, w_out, b_out, mode=KMODE):
    d = np.arange(D)
    perm_q = (192 * np.arange(H)[:, None] + d[None, :]).reshape(-1)
    perm = np.concatenate([perm_q, perm_q + 64, perm_q + 128])
    wqkvT = np.ascontiguousarray(w_qkv[perm, :].T, dtype=np.float32)
    bcols = np.ascontiguousarray(
        b_qkv[perm].reshape(24, 128).T, dtype=np.float32)
    # out-proj row perm: S2 row 128c+64dlt+d holds feature 64*(8dlt+c)+d
    co, dl = np.arange(8), np.arange(2)
    perm_o = (64 * (8 * dl[None, :, None] + co[:, None, None])
              + d[None, None, :]).reshape(-1)
    woutT = np.ascontiguousarray(w_out.T[perm_o, :], dtype=np.float32)
    borep = np.ascontiguousarray(
        np.broadcast_to(b_out[None, :], (128, C)), dtype=np.float32)
    maskB = np.tile((np.arange(128)[:, None] % 8
                     == np.arange(128)[None, :] % 8).astype(np.float32), (1, 4))
    xT = np.ascontiguousarray(x.T, dtype=np.float32)
    xT = _wcast(xT, mode)
    wqkvT = _wcast(wqkvT, mode)
    woutT = _wcast(woutT, mode)
    if mode == "bf16":
        maskB = np.ascontiguousarray(maskB.astype(ml_dtypes.bfloat16))
    return xT, wqkvT, bcols, woutT, borep, maskB


_cache = {}


def kernel(x, w_qkv, b_qkv, w_out, b_out, _trace=False, _tmpdir=None):
    x = np.asarray(x)
    n = x.shape[0]
    tok = n // N_CORES
    xT, wqkvT, bcols, woutT, borep, maskB = _host_prep(
        np.asarray(x), np.asarray(w_qkv), np.asarray(b_qkv),
        np.asarray(w_out), np.asarray(b_out))
    pipe = os.environ.get("PIPE", "1") == "1"
    key = (tok, KMODE, pipe)
    if key not in _cache:
        _cache[key] = build_pipe(tok) if pipe else build(tok)
    nc = _cache[key]
    shared = dict(wqkvT=wqkvT, woutT=woutT, bcols=bcols, borep=borep, maskB=maskB)
    in_maps = [dict(xT=np.ascontiguousarray(xT[:, i * tok:(i + 1) * tok]), **shared)
               for i in range(N_CORES)]
    res = run_bass_kernel_spmd(nc, in_maps, core_ids=list(range(N_CORES)),
                               trace=_trace, tmpdir=_tmpdir)
    out = np.concatenate([res.results[i]["out"] for i in range(N_CORES)], axis=0)
    kernel.last_results = res
    return out


# revision 23
# speedup vs baseline: 4514.6145x; 4514.6145x over previous
"""Trainium2 Bass kernel for nn_MultiHeadAttention_72189810312078.

Computation (per token): qkv = x @ w_qkv.T + b_qkv; per-token attention over
the 16 heads with 16x16 score matrices; out = attn_out @ w_out.T + b_out.

Strategy: data-parallel over 8 NeuronCores (8192 tokens each). Host
pre-transposes x to xT [1024, N] so the channel (contraction) dim lands on
SBUF partitions. Per core, per 256-token superblock:
  1) qkvT projection: 24 feature-chunk matmuls (N=256), K=1024 accumulated in
     PSUM; per-partition bias added in PSUM with one tensor_scalar_add.
  2) PSUM chunks scatter-evicted into attention staging: Q at partitions
     64:128 of T1, K at 64:128 / V at 0:64 of T2 (matmul operands need equal
     base partitions), laid out [d, (group, head, t)].
  3) Attention in groups of 8 tokens ((g,t) packs 16x8=128 partitions):
     scoresT = K.T @ Q per group (K=64 matmul at tile_position row 64);
     exp on ScalarE; multiplicative block-diagonal mask; V8 =
     PE-transpose(V); attnV matmul with a ones column appended to V8 so the
     softmax denominator falls out of the same matmul; normalize with a
     per-partition reciprocal scale on eviction.
  4) attn output PE-transposed back to feature-major, packed into S2
     [128 = (dlt,d), chunk x token]; out-projection against host-permuted
     w_out.T rows (feature 64*(8*dlt+c)+d at S2 row 128c+64*dlt+d); bias
     added from a replicated tile during eviction; result DMA'd row-major.

Dtype modes (KMODE): "bf16" (default) runs every matmul operand in bfloat16
(1 cycle/row on the PE at any moving size, FWL weight loads); "f32r" keeps
fp32-rounded (11-bit mantissa) operands for the projections (1 cycle/row only
at moving>=256) with fp32 attention staging. PSUM accumulation is fp32 always.
"""

import os
import sys
from contextlib import ExitStack, nullcontext

sys.path.insert(0, "/opt/trn_rl_repo")

import numpy as np
import ml_dtypes

import concourse.bass as bass  # noqa: E402
import concourse.bacc as bacc  # noqa: E402
import concourse.tile as tile  # noqa: E402
from concourse import mybir  # noqa: E402
from concourse.bass_utils import run_bass_kernel_spmd  # noqa: E402
from concourse.masks import make_identity  # noqa: E402

F32 = mybir.dt.float32
F32R = mybir.dt.float32r
BF16 = mybir.dt.bfloat16

N_CORES = 8
H, D, C = 16, 64, 1024
SB = 256   # tokens per superblock (projection moving dim)
SS = 128   # tokens per attention sub-stage / out-projection block
NG = SB // 8   # token groups per superblock (32)

KMODE = os.environ.get("KMODE", "f32r")
GPS = int(os.environ.get("GPS", "3"))  # bitmask: 1=memset, 2=mask-mul, 4=bias
Exp = mybir.ActivationFunctionType.Exp
Copy = mybir.ActivationFunctionType.Copy
Ident = mybir.ActivationFunctionType.Identity


def _dtypes(mode):
    """-> (WD projection-weight dtype, SD attention staging, AD S2)."""
    if mode == "bf16":
        return BF16, BF16, BF16
    if mode == "f32r":
        return F32R, F32R, F32R
    return F32, F32, F32


XBF = os.environ.get("XBF", "0") == "1"   # x streamed in bf16 (moving operand)


def build(tok, mode=KMODE, static_loop=False):
    WD, SD, AD = _dtypes(mode)

    nc = bacc.Bacc("TRN2", target_bir_lowering=False, debug=False,
                   enable_asserts=True, num_devices=N_CORES)
    xT_d = nc.dram_tensor("xT", [C, tok], WD, kind="ExternalInput").ap()
    wqkvT_d = nc.dram_tensor("wqkvT", [C, 3 * C], WD, kind="ExternalInput").ap()
    woutT_d = nc.dram_tensor("woutT", [C, C], WD, kind="ExternalInput").ap()
    bcols_d = nc.dram_tensor("bcols", [128, 24], F32, kind="ExternalInput").ap()
    borep_d = nc.dram_tensor("borep", [128, C], F32, kind="ExternalInput").ap()
    maskB_d = nc.dram_tensor("maskB", [128, 512], SD, kind="ExternalInput").ap()
    out_d = nc.dram_tensor("out", [tok, C], F32, kind="ExternalOutput").ap()

    with tile.TileContext(nc) as tc, ExitStack() as ctx:
        consts = ctx.enter_context(tc.tile_pool(name="consts", bufs=1))
        xin = ctx.enter_context(tc.tile_pool(name="xin", bufs=2))
        stag = ctx.enter_context(tc.tile_pool(name="stag", bufs=1))
        smx = ctx.enter_context(tc.tile_pool(name="smx", bufs=2))
        s2p = ctx.enter_context(tc.tile_pool(name="s2p", bufs=2))
        outp = ctx.enter_context(tc.tile_pool(name="outp", bufs=2))
        psA = ctx.enter_context(tc.tile_pool(name="psA", bufs=2, space="PSUM"))
        psSp = ctx.enter_context(tc.tile_pool(name="psSp", bufs=2, space="PSUM"))
        psVp = ctx.enter_context(tc.tile_pool(name="psVp", bufs=1, space="PSUM"))
        psC2p = ctx.enter_context(tc.tile_pool(name="psC2p", bufs=1, space="PSUM"))
        psTp = ctx.enter_context(tc.tile_pool(name="psTp", bufs=1, space="PSUM"))
        psOp = ctx.enter_context(tc.tile_pool(name="psOp", bufs=1, space="PSUM"))

        # ---- constants ----
        wq_sb = consts.tile([128, 8, 3 * C], WD)
        nc.sync.dma_start(out=wq_sb, in_=wqkvT_d.rearrange("(ci p) f -> p ci f", p=128))
        wo_sb = consts.tile([128, 8, C], WD)
        nc.sync.dma_start(out=wo_sb, in_=woutT_d.rearrange("(ci p) f -> p ci f", p=128))
        bcols_sb = consts.tile([128, 24], F32)
        nc.sync.dma_start(out=bcols_sb, in_=bcols_d)
        borep_sb = consts.tile([128, C], F32)
        nc.sync.dma_start(out=borep_sb, in_=borep_d)
        maskB_sb = consts.tile([128, 512], SD)
        nc.sync.dma_start(out=maskB_sb, in_=maskB_d)
        idq = consts.tile([128, 128], F32)
        make_identity(nc, idq)
        if SD is F32:
            idS = idq
        else:
            idS = consts.tile([128, 128], SD)
            nc.vector.tensor_copy(idS, idq)
        if AD is F32:
            idr = idq
        elif AD is SD:
            idr = idS
        else:
            idr = consts.tile([128, 128], AD)
            nc.vector.tensor_copy(idr, idq)

        ecnt = 0  # evict-engine round robin

        def evict_copy(dst, src):
            nonlocal ecnt
            if ecnt % 2 == 0:
                nc.vector.tensor_copy(dst, src)
            else:
                nc.scalar.copy(dst, src)
            ecnt += 1

        xT_r = xT_d.rearrange("(ci p) t -> p ci t", p=128)
        if static_loop:
            loop_iter = [(nullcontext(iv), iv) for iv in range(0, tok, SB)]
        else:
            fc = tc.For_i(0, tok, SB,
                          hint_engines=(mybir.EngineType.PE,
                                        mybir.EngineType.DVE))
            loop_iter = [(fc, None)]
        for _ctx, _iv in loop_iter:
          with _ctx as _cv:
            iv = _iv if _iv is not None else _cv
            x_sb = xin.tile([128, 8, SB], WD)
            nc.sync.dma_start(out=x_sb, in_=xT_r[:, :, bass.ds(iv, SB)])

            # staging: T1 rows 64:128 = Q; T2 rows 64:128 = K, rows 0:64 = V
            T1 = stag.tile([128, NG, 16, 8], SD, name="T1")
            T2 = stag.tile([128, NG, 16, 8], SD, name="T2")

            # ---- qkv projection + scatter-evict (bias fused / on gpsimd) ----
            for co in range(24):
                psC1 = psA.tile([128, SB], F32)
                for ci in range(8):
                    nc.tensor.matmul(psC1, wq_sb[:, ci, co * 128:(co + 1) * 128],
                                     x_sb[:, ci, :], start=(ci == 0),
                                     stop=(ci == 7))
                kind, c = co // 8, co % 8
                for dlt in range(2):
                    src = psC1[64 * dlt:64 * dlt + 64, :].rearrange(
                        "p (g t) -> p g t", g=NG)
                    hslot = 2 * c + dlt
                    if kind == 0:
                        dst = T1[64:128, :, hslot, :]
                    elif kind == 1:
                        dst = T2[64:128, :, hslot, :]
                    else:
                        dst = T2[0:64, :, hslot, :]
                    bias = bcols_sb[64 * dlt:64 * dlt + 64, co:co + 1]
                    if dlt == 0:
                        # DVE evict with fused bias add
                        nc.vector.tensor_scalar_add(dst, src, bias)
                    elif GPS & 4:
                        # ACT plain evict, bias added SBUF-side on idle gpsimd
                        nc.scalar.copy(dst, src)
                        nc.gpsimd.tensor_scalar_add(dst, dst, bias)
                    else:
                        nc.vector.tensor_scalar_add(dst, src, bias)

            # ---- attention (8 batches of 4 groups) + out-proj per 128 tok ----
            for iss in range(2):
                S2 = s2p.tile([128, 8, SS], AD)
                for b4 in range(4 * iss, 4 * iss + 4):
                    psS = psSp.tile([128, 512], F32)
                    psV = psVp.tile([128, 4, 64], SD)
                    for j in range(4):
                        g = 4 * b4 + j
                        nc.tensor.matmul(psS[:, 128 * j:128 * j + 128],
                                         T2[64:128, g, :, :], T1[64:128, g, :, :],
                                         start=True, stop=True)
                        nc.tensor.transpose(psV[:, j, :], T2[0:64, g, :, :],
                                            idS[0:64, 0:64])
                    es4 = smx.tile([128, 512], SD)
                    nc.scalar.activation(es4, psS, Exp, scale=0.125)
                    if GPS & 2:
                        nc.gpsimd.tensor_mul(es4, es4, maskB_sb)
                    else:
                        nc.vector.tensor_mul(es4, es4, maskB_sb)
                    V8sb = smx.tile([128, 4, 66], SD)
                    nc.scalar.copy(V8sb[:, :, 0:64], psV)
                    if GPS & 1:
                        nc.gpsimd.memset(V8sb[:, :, 64:65], 1.0)
                    else:
                        nc.vector.memset(V8sb[:, :, 64:65], 1.0)
                    psC2 = psC2p.tile([128, 4, 66], F32)
                    for j in range(4):
                        nc.tensor.matmul(psC2[:, j, 0:65],
                                         es4[:, 128 * j:128 * j + 128],
                                         V8sb[:, j, 0:65], start=True, stop=True)
                    rec4 = smx.tile([128, 4], F32)
                    nc.vector.reciprocal(rec4, psC2[:, :, 64:65])
                    attno = smx.tile([128, 4, 64], AD)
                    for j in range(4):
                        if j % 2 == 0:
                            nc.scalar.activation(attno[:, j, :], psC2[:, j, 0:64],
                                                 Copy, scale=rec4[:, j:j + 1])
                        else:
                            nc.vector.tensor_scalar_mul(attno[:, j, :],
                                                        psC2[:, j, 0:64],
                                                        rec4[:, j:j + 1])
                    psT = psTp.tile([64, 4, 128], AD)
                    for j in range(4):
                        nc.tensor.transpose(psT[:, j, :], attno[:, j, :], idr)
                    # S2 pack: head slots 8*dlt..8*dlt+7 -> S2 rows 64*dlt+d
                    for dlt in range(2):
                        src = psT[:, :, 64 * dlt:64 * dlt + 64].rearrange(
                            "p j (h t) -> p j h t", h=8)
                        dst = S2[64 * dlt:64 * dlt + 64].rearrange(
                            "p c (gb gj t) -> p gj c gb t", gb=4, gj=4)[
                                :, :, :, b4 % 4, :]
                        evict_copy(dst, src)

                # ---- out-projection for this 128-token block ----
                outsb = outp.tile([128, C], F32)
                for nh in range(2):
                    psO = psOp.tile([128, 512], F32)
                    for c in range(8):
                        nc.tensor.matmul(psO, S2[:, c, :],
                                         wo_sb[:, c, 512 * nh:512 * nh + 512],
                                         start=(c == 0), stop=(c == 7))
                    nc.vector.tensor_add(outsb[:, 512 * nh:512 * nh + 512], psO,
                                         borep_sb[:, 512 * nh:512 * nh + 512])
                nc.sync.dma_start(out=out_d[bass.ds(iv + SS * iss, SS), :],
                                  in_=outsb)

    nc.compile()
    return nc


def build_pipe(tok, mode=KMODE, static_loop=False, reps=1):
    """Software-pipelined build v3: all-f32r matmuls (self-loading weights, no
    standalone LDWEIGHTS), attention via group-PAIR matmuls so every PE op has
    a 256-wide moving operand (f32r fast path):
      - scores: per pair (gA,gB), two matmuls K_g^T @ [Q_gA|Q_gB] (256 moving)
      - exp on ScalarE -> es (bf16); pair mask (kills cross-group and
        cross-token terms) on GpSimd
      - attnV flipped: psF[d(+Z row), QpairCols] = sum_X V8_X^T @ es_X with the
        V8 ones-column producing the softmax denominator row; S2 packed
        directly from psF (no output transpose)
      - denominators: Z row gathered to zbuf, PE-transposed to per-token
        column, reciprocal on DVE, applied as per-partition scale during the
        out-projection eviction on ScalarE.
    Projection of superblock k+1 overlaps attention of superblock k."""
    WD, SD, AD = _dtypes(mode)
    XD = BF16 if XBF else WD

    nc = bacc.Bacc("TRN2", target_bir_lowering=False, debug=False,
                   enable_asserts=True, num_devices=N_CORES)
    xT_d = nc.dram_tensor("xT", [C, tok], XD, kind="ExternalInput").ap()
    wqkvT_d = nc.dram_tensor("wqkvT", [C, 3 * C], WD, kind="ExternalInput").ap()
    woutT_d = nc.dram_tensor("woutT", [C, C], WD, kind="ExternalInput").ap()
    bcols_d = nc.dram_tensor("bcols", [128, 24], F32, kind="ExternalInput").ap()
    borep_d = nc.dram_tensor("borep", [128, C], BF16, kind="ExternalInput").ap()
    maskP_d = nc.dram_tensor("maskP", [128, 128], BF16, kind="ExternalInput").ap()
    out_d = nc.dram_tensor("out", [tok, C], F32, kind="ExternalOutput").ap()

    PB = [int(v) for v in os.environ.get("PB", "1,2,2,2,2,2").split(",")]
    with tile.TileContext(nc) as tc, ExitStack() as ctx:
        consts = ctx.enter_context(tc.tile_pool(name="consts", bufs=1))
        xin = ctx.enter_context(tc.tile_pool(name="xin", bufs=PB[0]))
        stag = ctx.enter_context(tc.tile_pool(name="stag", bufs=1))
        smx = ctx.enter_context(tc.tile_pool(name="smx", bufs=PB[1]))
        s2p = ctx.enter_context(tc.tile_pool(name="s2p", bufs=PB[2]))
        outp = ctx.enter_context(tc.tile_pool(name="outp", bufs=PB[3]))
        psA = ctx.enter_context(tc.tile_pool(name="psA", bufs=PB[4], space="PSUM"))
        psSp = ctx.enter_context(tc.tile_pool(name="psSp", bufs=PB[5], space="PSUM"))
        psVp = ctx.enter_context(tc.tile_pool(name="psVp", bufs=1, space="PSUM"))
        psFp = ctx.enter_context(tc.tile_pool(name="psFp", bufs=2, space="PSUM"))
        psOp = ctx.enter_context(tc.tile_pool(name="psOp", bufs=1, space="PSUM"))

        wq_sb = consts.tile([128, 8, 3 * C], WD)
        nc.sync.dma_start(out=wq_sb, in_=wqkvT_d.rearrange("(ci p) f -> p ci f", p=128))
        wo_sb = consts.tile([128, 8, C], WD)
        nc.sync.dma_start(out=wo_sb, in_=woutT_d.rearrange("(ci p) f -> p ci f", p=128))
        bcols_sb = consts.tile([128, 24], F32)
        nc.sync.dma_start(out=bcols_sb, in_=bcols_d)
        borep_sb = consts.tile([128, C], BF16)
        nc.sync.dma_start(out=borep_sb, in_=borep_d)
        maskT_sb = consts.tile([128, 128], BF16)
        nc.sync.dma_start(out=maskT_sb, in_=maskP_d)
        idq = consts.tile([128, 128], F32)
        make_identity(nc, idq)
        if SD is F32:
            idS = idq
        else:
            idS = consts.tile([128, 128], SD)
            nc.vector.tensor_copy(idS, idq)

        # persistent ping-pong staging (half-set hb=0: Q/K upper, V lower)
        Q_AB = stag.tile([128, NG, 16, 8], SD, name="Q_AB")
        K_AB = stag.tile([128, NG, 16, 8], SD, name="K_AB")
        V_AB = stag.tile([128, NG, 16, 8], SD, name="V_AB")
        # persistent es pair tiles [128, X, gp, 16, 8]; the gp != X (cross
        # group) halves are zeroed once here and never written again, so the
        # attnV pair matmuls read zeros there without any recurring masking.
        es_pp = [stag.tile([128, 2, 2, 16, 8], SD, name=f"es{i}")
                 for i in range(2)]
        zeroC = consts.tile([128, 128], BF16)
        nc.vector.memset(zeroC, 0.0)
        onesC = consts.tile([128, 64], BF16)
        nc.vector.memset(onesC, 1.0)
        for e in es_pp:
            nc.vector.tensor_copy(e[:, 0, 1].rearrange("p h t -> p (h t)"), zeroC)
            nc.vector.tensor_copy(e[:, 1, 0].rearrange("p h t -> p (h t)"), zeroC)
        # persistent V8 pair tiles [128, X, 128]: cols 0:64 = V^T (rewritten
        # each pair), cols 64:128 = constant ones so the attnV matmul output
        # rows 64:128 replicate the softmax-denominator row across partitions
        v8_pp = [stag.tile([128, 2, 128], SD, name=f"v8_{i}") for i in range(2)]
        for e in v8_pp:
            nc.vector.tensor_copy(e[:, 0, 64:128], onesC)
            nc.vector.tensor_copy(e[:, 1, 64:128], onesC)

        xT_r = xT_d.rearrange("(ci p) t -> p ci t", p=128)

        def emit_xload(piv):
            x_sb = xin.tile([128, 8, SB], XD)
            nc.sync.dma_start(out=x_sb, in_=xT_r[:, :, bass.ds(piv, SB)])
            return x_sb

        def qk_half(T, hb):
            return T[64 * (1 - hb):64 * (1 - hb) + 64]

        def v_half(hb):
            return V_AB[64 * hb:64 * hb + 64]

        def emit_proj_chunk(x_sb, co, hb):
            psC1 = psA.tile([128, SB], F32)
            for ci in range(8):
                nc.tensor.matmul(psC1, wq_sb[:, ci, co * 128:(co + 1) * 128],
                                 x_sb[:, ci, :], start=(ci == 0), stop=(ci == 7))
            kind, c = co // 8, co % 8
            for dlt in range(2):
                src = psC1[64 * dlt:64 * dlt + 64, :].rearrange(
                    "p (g t) -> p g t", g=NG)
                hslot = 2 * c + dlt
                if kind == 0:
                    dst = qk_half(Q_AB, hb)[:, :, hslot, :]
                elif kind == 1:
                    dst = qk_half(K_AB, hb)[:, :, hslot, :]
                else:
                    dst = v_half(hb)[:, :, hslot, :]
                bias = bcols_sb[64 * dlt:64 * dlt + 64, co:co + 1]
                if (co + dlt) % 2 == 0:
                    nc.vector.tensor_scalar_add(dst, src, bias)
                else:
                    nc.scalar.activation(dst, src, Ident, bias=bias)

        def emit_attn_batch1(b4, hb):
            """scores (pair matmuls) + V transposes + exp + pair-mask for the
            two pairs of batch b4 (groups 4*b4..4*b4+3)."""
            vb = 64 * hb
            pend = []
            for p in range(2):
                gA = 4 * b4 + 2 * p
                psS = psSp.tile([128, 2, 256], F32)
                psV = psVp.tile([128, 2, 64], SD)
                qpair = qk_half(Q_AB, hb)[:, gA:gA + 2, :, :]
                for X in range(2):
                    nc.tensor.matmul(psS[:, X, :],
                                     qk_half(K_AB, hb)[:, gA + X, :, :],
                                     qpair, start=True, stop=True)
                    nc.tensor.transpose(psV[:, X, :], v_half(hb)[:, gA + X, :, :],
                                        idS[vb:vb + 64, vb:vb + 64])
                es = es_pp[p]
                for X in range(2):
                    nc.scalar.activation(es[:, X, X],
                                         psS[:, X, 128 * X:128 * X + 128],
                                         Exp, scale=0.125)
                nc.gpsimd.tensor_mul(es[:, 0, 0], es[:, 0, 0], maskT_sb)
                nc.gpsimd.tensor_mul(es[:, 1, 1], es[:, 1, 1], maskT_sb)
                V8sb = v8_pp[p]
                if p == 0:
                    nc.vector.tensor_copy(V8sb[:, :, 0:64], psV)
                else:
                    nc.scalar.copy(V8sb[:, :, 0:64], psV)
                pend.append((es, V8sb))
            return pend

        def emit_attn_batch2(b4, pend, S2):
            for p in range(2):
                es, V8sb = pend[p]
                psF = psFp.tile([128, 2, 16, 8], F32)
                esf = es.rearrange("q x gp h t -> q x (gp h t)")
                for X in range(2):
                    nc.tensor.matmul(psF, V8sb[:, X, :], esf[:, X, :],
                                     start=(X == 0), stop=(X == 1))
                # psF rows 64:128 hold the per-(head, token) softmax
                # denominator row replicated by the ones columns of V8
                rZB = smx.tile([64, 2, 16, 8], F32, name="rZB")
                nc.vector.reciprocal(rZB, psF[64:128])
                for dlt in range(2):
                    src = psF[0:64].rearrange("p gp h t -> p h gp t")[
                        :, 8 * dlt:8 * dlt + 8, :, :]
                    rzs = rZB.rearrange("p gp h t -> p h gp t")[
                        :, 8 * dlt:8 * dlt + 8, :, :]
                    dst = S2[64 * dlt:64 * dlt + 64].rearrange(
                        "p c (gb pp gp t) -> p c gb pp gp t",
                        gb=4, pp=2, gp=2)[:, :, b4 % 4, p, :, :]
                    nc.vector.tensor_mul(dst, src, rzs)

        def emit_outproj(S2, oiv, iss):
            for nh in range(2):
                psO = psOp.tile([128, 512], F32)
                for c in range(8):
                    nc.tensor.matmul(psO, S2[:, c, :],
                                     wo_sb[:, c, 512 * nh:512 * nh + 512],
                                     start=(c == 0), stop=(c == 7))
                outsb = outp.tile([128, 512], F32, name="outsb")
                nc.scalar.copy(outsb, psO)
                nc.gpsimd.tensor_add(outsb, outsb,
                                     borep_sb[:, 512 * nh:512 * nh + 512])
                nc.sync.dma_start(
                    out=out_d[bass.ds(oiv + SS * iss, SS),
                              bass.ds(512 * nh, 512)], in_=outsb)

        def emit_part(attn_oiv, attn_hb, proj_piv, proj_hb):
            """Weave attention of one superblock with projection of another.
            Either may be None (prologue/epilogue)."""
            x_sb = emit_xload(proj_piv) if proj_piv is not None else None
            S2 = None
            dpo = None   # deferred out-projection (S2, iss)
            for b4 in range(8):
                if attn_oiv is not None:
                    if b4 % 4 == 0:
                        S2 = s2p.tile([128, 8, SS], AD, name="S2")
                    pend = emit_attn_batch1(b4, attn_hb)
                    if dpo is not None:
                        emit_outproj(dpo[0], attn_oiv, dpo[1])
                        dpo = None
                if x_sb is not None:
                    for co in range(4 * b4, min(4 * b4 + 4, 24)):
                        emit_proj_chunk(x_sb, co, proj_hb)
                if attn_oiv is not None:
                    emit_attn_batch2(b4, pend, S2)
                    if b4 % 4 == 3:
                        dpo = (S2, b4 // 4)
            if dpo is not None:
                emit_outproj(dpo[0], attn_oiv, dpo[1])

        assert tok % (2 * SB) == 0 and tok >= 2 * SB
        emit_part(None, None, 0, 0)                      # prologue: proj sb0 -> A
        if tok > 2 * SB and static_loop:
            for iv in range(0, tok - 2 * SB, 2 * SB):
                emit_part(iv, 0, iv + SB, 1)             # attn A, proj -> B
                emit_part(iv + SB, 1, iv + 2 * SB, 0)    # attn B, proj -> A
        elif tok > 2 * SB and reps == 1:
            with tc.For_i(0, tok - 2 * SB, 2 * SB,
                          hint_engines=(mybir.EngineType.PE, mybir.EngineType.DVE,
                                        mybir.EngineType.Activation)) as iv:
                emit_part(iv, 0, iv + SB, 1)             # attn A, proj -> B
                emit_part(iv + SB, 1, iv + 2 * SB, 0)    # attn B, proj -> A
        elif tok > 2 * SB:
            with tc.For_i(0, reps, 1) as _rep:
                with tc.For_i(0, tok - 2 * SB, 2 * SB,
                              hint_engines=(mybir.EngineType.PE,
                                            mybir.EngineType.DVE,
                                            mybir.EngineType.Activation)) as iv:
                    emit_part(iv, 0, iv + SB, 1)         # attn A, proj -> B
                    emit_part(iv + SB, 1, iv + 2 * SB, 0)  # attn B, proj -> A
        last = tok - 2 * SB
        emit_part(last, 0, tok - SB, 1)                  # attn A, proj last -> B
        emit_part(tok - SB, 1, None, None)               # attn B
    nc.compile()
    return nc


def _round_f32r(a):
    """Round fp32 to the f32r grid (drop 12 mantissa bits, round-to-nearest)."""
    b = np.ascontiguousarray(a, dtype=np.float32).view(np.uint32)
    b = ((b + (1 << 11)) >> 12) << 12
    return b.view(np.float32)


def _wcast(a, mode):
    if mode == "bf16":
        return np.ascontiguousarray(a.astype(ml_dtypes.bfloat16))
    if mode == "f32r":
        return _round_f32r(np.ascontiguousarray(a, dtype=np.float32))
    return np.ascontiguousarray(a, dtype=np.float32)


def _host_prep(x, w_qkv, b_qkv, w_out, b_out, mode=KMODE):
    d = np.arange(D)
    perm_q = (192 * np.arange(H)[:, None] + d[None, :]).reshape(-1)
    perm = np.concatenate([perm_q, perm_q + 64, perm_q + 128])
    wqkvT = np.ascontiguousarray(w_qkv[perm, :].T, dtype=np.float32)
    bcols = np.ascontiguousarray(
        b_qkv[perm].reshape(24, 128).T, dtype=np.float32)
    # out-proj row perm: S2 row 128c+64dlt+d holds feature 64*(8dlt+c)+d
    co, dl = np.arange(8), np.arange(2)
    perm_o = (64 * (8 * dl[None, :, None] + co[:, None, None])
              + d[None, None, :]).reshape(-1)
    woutT = np.ascontiguousarray(w_out.T[perm_o, :], dtype=np.float32)
    borep = np.ascontiguousarray(
        np.broadcast_to(b_out[None, :], (128, C)), dtype=np.float32)
    maskB = np.tile((np.arange(128)[:, None] % 8
                     == np.arange(128)[None, :] % 8).astype(np.float32), (1, 4))
    # in-group mask [128 rows=(hk,tk), (hq, tq)]: keep tk==tq
    maskP = np.ascontiguousarray(
        (np.arange(128)[:, None] % 8 == np.arange(128)[None, :] % 8
         ).astype(ml_dtypes.bfloat16))
    borep16 = np.ascontiguousarray(borep.astype(ml_dtypes.bfloat16))
    xT = np.ascontiguousarray(x.T, dtype=np.float32)
    if XBF:
        xT16 = np.ascontiguousarray(xT.astype(ml_dtypes.bfloat16))
    else:
        xT16 = _wcast(xT, mode)
    xT = _wcast(xT, mode)
    wqkvT = _wcast(wqkvT, mode)
    woutT = _wcast(woutT, mode)
    if mode == "bf16":
        maskB = np.ascontiguousarray(maskB.astype(ml_dtypes.bfloat16))
    return dict(xT=xT, xT16=xT16, wqkvT=wqkvT, bcols=bcols, woutT=woutT,
                borep=borep, borep16=borep16, maskB=maskB, maskP=maskP)


_cache = {}


def kernel(x, w_qkv, b_qkv, w_out, b_out, _trace=False, _tmpdir=None):
    x = np.asarray(x)
    n = x.shape[0]
    tok = n // N_CORES
    hp = _host_prep(
        np.asarray(x), np.asarray(w_qkv), np.asarray(b_qkv),
        np.asarray(w_out), np.asarray(b_out))
    pipe = os.environ.get("PIPE", "1") == "1"
    key = (tok, KMODE, pipe)
    if key not in _cache:
        _cache[key] = build_pipe(tok) if pipe else build(tok)
    nc = _cache[key]
    if pipe:
        xT = hp["xT16"]
        shared = dict(wqkvT=hp["wqkvT"], woutT=hp["woutT"], bcols=hp["bcols"],
                      borep=hp["borep16"], maskP=hp["maskP"])
    else:
        xT = hp["xT"]
        shared = dict(wqkvT=hp["wqkvT"], woutT=hp["woutT"], bcols=hp["bcols"],
                      borep=hp["borep"], maskB=hp["maskB"])
    in_maps = [dict(xT=np.ascontiguousarray(xT[:, i * tok:(i + 1) * tok]), **shared)
               for i in range(N_CORES)]
    res = run_bass_kernel_spmd(nc, in_maps, core_ids=list(range(N_CORES)),
                               trace=_trace, tmpdir=_tmpdir)
    out = np.concatenate([res.results[i]["out"] for i in range(N_CORES)], axis=0)
    kernel.last_results = res
    return out
